# revision 25
# baseline (speedup 1.0000x reference)
"""BitNet transformer block on 8 Trainium2 NeuronCores — v2.

Hybrid tensor/sequence parallel chosen from HW-measured collective costs
(every collective costs ~80-200us regardless of size, so the design
minimizes collective COUNT; only shared-output 8-core AllGather and one
small AllToAll are used — no AllReduce/ReduceScatter):

  1. Each core LN1+quantizes its own 512 tokens; ONE AllGather ships the
     quantized tokens (bf16 ints, feature-major) with the fp32 token
     scales and the per-core |W| partial sums bit-packed into 2 extra
     bf16 rows.
  2. QKV + attention are head-parallel (2 heads x 2 batches per core),
     so K/V stay local.  Scores run in fp32r on integer Q/K (exact);
     V is scale-folded in fp32 and split hi/lo bf16 (exact to ~1e-5) so
     only the bf16 probs rounding remains (~4e-3).
  3. One 4MB fp32 AllToAll re-shards the attention output to
     token-sharded full-D, so the o-quant absmax is local.
  4. Out-proj and the whole FFN run sequence-parallel with FULL ternary
     weights delivered by three AllGathers (wo/wu/wd) issued early and
     overlapped with attention compute.  gelu stays fp32 (DRAM spill)
     and is re-quantized from fp32, eliminating bf16 requant error.

Weights arrive host-side pre-transposed/packed (layout only) so zero
weight transposes happen on device; activation transposes use multi-tile
XBAR transpose-DMA instructions (one per 128-row tile group).
"""

import numpy as np
from contextlib import ExitStack

B, T, D, H, HD, F = 2, 2048, 2048, 16, 128, 8192
NCORES = 8
TPC = 512            # tokens per core
MAGIC = 12582912.0   # 1.5*2**23: +M then -M rounds f32 to nearest-even int
INV127 = float(np.float32(1.0) / np.float32(127.0))
SMSCALE = float(np.float32(1.0) / np.sqrt(np.float32(HD)))
NEG = -1.0e30


def build_graph():
    import concourse.bass as bass
    import concourse.bacc as bacc
    import concourse.tile as tile
    from concourse import mybir, bass_isa

    f32 = mybir.dt.float32
    bf16 = mybir.dt.bfloat16
    f32r = mybir.dt.float32r
    Alu = mybir.AluOpType
    Act = mybir.ActivationFunctionType
    Ax = mybir.AxisListType
    Red = bass_isa.ReduceOp

    nc = bacc.Bacc("TRN2", target_bir_lowering=False, debug=False,
                   num_devices=NCORES)

    x_d = nc.dram_tensor("x", [TPC, D], f32, kind="ExternalInput").ap()
    xT_d = nc.dram_tensor("xT", [D, TPC], f32, kind="ExternalInput").ap()
    wqT_d = nc.dram_tensor("wqT", [128, 4096], f32, kind="ExternalInput").ap()
    wkT_d = nc.dram_tensor("wkT", [128, 4096], f32, kind="ExternalInput").ap()
    wvT_d = nc.dram_tensor("wvT", [128, 4096], f32, kind="ExternalInput").ap()
    woT_d = nc.dram_tensor("woT", [128, 4096], f32, kind="ExternalInput").ap()
    wuT_d = nc.dram_tensor("wuT", [128, 16384], f32,
                           kind="ExternalInput").ap()
    wdT_d = nc.dram_tensor("wdT", [128, 16384], f32,
                           kind="ExternalInput").ap()
    mask_d = nc.dram_tensor("mask", [4, 128, 512], f32,
                            kind="ExternalInput").ap()
    out_d = nc.dram_tensor("out", [D, TPC], f32, kind="ExternalOutput").ap()

    r_all = [list(range(NCORES))]

    with tile.TileContext(nc) as tc, ExitStack() as g:
        dram = g.enter_context(tc.tile_pool(name="dram", bufs=1,
                                            space="DRAM"))
        p0 = g.enter_context(tc.tile_pool(name="p0", bufs=1))
        stats = g.enter_context(tc.tile_pool(name="stats", bufs=2))
        perm = g.enter_context(tc.tile_pool(name="perm", bufs=2))

        # ---- DRAM buffers ----
        ag1_in = dram.tile([130, 8192], bf16, name="ag1_in")
        ag1_out = dram.tile([130 * NCORES, 8192], bf16, name="ag1_out",
                            addr_space="Shared")
        agwo_in = dram.tile([256, 2048], bf16, name="agwo_in")
        agwo_out = dram.tile([2048, 2048], bf16, name="agwo_out",
                             addr_space="Shared")
        agwu_in = dram.tile([256, 8192], bf16, name="agwu_in")
        agwu_out = dram.tile([2048, 8192], bf16, name="agwu_out",
                             addr_space="Shared")
        agwd_in = dram.tile([256, 8192], bf16, name="agwd_in")
        agwd_out = dram.tile([2048, 8192], bf16, name="agwd_out",
                             addr_space="Shared")
        a2a_in = dram.tile([2048, TPC], f32, name="a2a_in")
        a2a_out = dram.tile([2048, TPC], f32, name="a2a_out")
        x2t_dram = dram.tile([D, TPC], f32, name="x2t_dram")
        rse_dram = [dram.tile([1, 512], f32, name=f"rse_dram{i}")
                    for i in range(2)]
        u_dram = dram.tile([F, TPC], f32, name="u_dram")

        eps_t = p0.tile([128, 1], f32, name="eps")
        nc.vector.memset(eps_t, 1.0e-5)
        scaleA = [p0.tile([128, 1], f32, name=f"sA{i}") for i in range(4)]

        def ln(src, dst):
            st = stats.tile([128, 4, 6], f32, tag="bn", name="bn")
            for sg in range(4):
                nc.vector.bn_stats(out=st[:, sg, :],
                                   in_=src[:, sg * 512:(sg + 1) * 512])
            mv = stats.tile([128, 2], f32, tag="mv", name="mv")
            nc.vector.bn_aggr(out=mv, in_=st)
            sq = stats.tile([128, 1], f32, tag="sq", name="sq")
            nc.scalar.activation(out=sq, in_=mv[:, 1:2], func=Act.Sqrt,
                                 bias=eps_t, scale=1.0)
            rstd = stats.tile([128, 1], f32, tag="rstd", name="rstd")
            nc.vector.reciprocal(out=rstd, in_=sq)
            nc.vector.tensor_scalar(out=dst, in0=src, scalar1=mv[:, 0:1],
                                    scalar2=rstd, op0=Alu.subtract,
                                    op1=Alu.mult)

        def quant(h, xq_out, scale_out):
            # NOTE: destroys h (rounding pass is in-place)
            amax = stats.tile([128, 1], f32, tag="amax", name="amax")
            nc.vector.tensor_reduce(out=amax, in_=h, axis=Ax.X, op=Alu.max,
                                    apply_absolute_value=True)
            nc.vector.tensor_scalar(out=scale_out, in0=amax, scalar1=INV127,
                                    scalar2=None, op0=Alu.mult)
            rq = stats.tile([128, 1], f32, tag="rq", name="rq")
            nc.vector.tensor_scalar(out=rq, in0=scale_out, scalar1=1e-8,
                                    scalar2=None, op0=Alu.add)
            nc.vector.reciprocal(out=rq, in_=rq)
            nc.vector.tensor_scalar(out=h, in0=h, scalar1=rq, scalar2=MAGIC,
                                    op0=Alu.mult, op1=Alu.add)
            nc.vector.tensor_scalar(out=xq_out, in0=h, scalar1=MAGIC,
                                    scalar2=None, op0=Alu.subtract)

        def row_scales(pool, row_in, rq_bc, sc_bc):
            """amax row [1,512] -> scale row + reciprocal, broadcast."""
            sc_row = pool.tile([1, 512], f32, tag="scrow", name="scrow")
            nc.vector.tensor_scalar(out=sc_row, in0=row_in, scalar1=INV127,
                                    scalar2=None, op0=Alu.mult)
            rq_row = pool.tile([1, 512], f32, tag="rqrow", name="rqrow")
            nc.vector.tensor_scalar(out=rq_row, in0=sc_row, scalar1=1e-8,
                                    scalar2=None, op0=Alu.add)
            nc.vector.reciprocal(out=rq_row, in_=rq_row)
            nc.gpsimd.partition_broadcast(rq_bc[:], rq_row[0:1, :],
                                          channels=128)
            nc.gpsimd.partition_broadcast(sc_bc[:], sc_row[0:1, :],
                                          channels=128)

        # scope nesting (LIFO): aw2 [P0..P3] > {aw1 [P0..P1], a0 [P2..P3]}
        cqs = [perm.tile([128, 16], f32, name=f"cqs{b}") for b in range(2)]
        aw2 = ExitStack()
        awp2 = aw2.enter_context(tc.tile_pool(name="awp2", bufs=1))
        scbcp = aw2.enter_context(tc.tile_pool(name="scbcp", bufs=1))
        aw1 = ExitStack()
        awp = aw1.enter_context(tc.tile_pool(name="awp", bufs=1))
        wchkp = aw1.enter_context(tc.tile_pool(name="wchk", bufs=2))

        # ---- P0: |W| stats (streamed in 1MB chunks) + LN1/quant + AG1
        srcs = [("wq", wqT_d, 2), ("wk", wkT_d, 2), ("wv", wvT_d, 2),
                ("wo", woT_d, 2), ("wu", wuT_d, 8), ("wd", wdT_d, 8)]
        pp = awp.tile([128, 24], f32, name="wpart")
        col = 0
        for nm, d_, nch_ in srcs:
            for c in range(nch_):
                t = wchkp.tile([128, 2048], f32, tag="wchk", name="wchk")
                nc.gpsimd.dma_start(out=t,
                                    in_=d_[:, c * 2048:(c + 1) * 2048])
                nc.vector.tensor_reduce(out=pp[:, col:col + 1], in_=t,
                                        axis=Ax.X, op=Alu.add,
                                        apply_absolute_value=True)
                col += 1
        ppr = awp.tile([128, 24], f32, name="wpartr")
        nc.gpsimd.partition_all_reduce(ppr[:], pp[:], channels=128,
                                       reduce_op=Red.add)
        wsp = awp.tile([1, 8], f32, name="wsp")
        col = 0
        for j, (nm, d_, nch_) in enumerate(srcs):
            nc.vector.tensor_reduce(out=wsp[0:1, j:j + 1],
                                    in_=ppr[0:1, col:col + nch_],
                                    axis=Ax.X, op=Alu.add)
            col += nch_

        with ExitStack() as s0:
            xp = s0.enter_context(tc.tile_pool(name="xp", bufs=2))
            h1p = s0.enter_context(tc.tile_pool(name="h1p", bufs=2))
            h1tp = s0.enter_context(tc.tile_pool(name="h1tp", bufs=1))
            h1t = h1tp.tile([128, 16, 4, 128], bf16, name="h1t")
            for tt in range(4):
                xt = xp.tile([128, D], f32, tag="xt", name="xt")
                nc.sync.dma_start(out=xt,
                                  in_=x_d[tt * 128:(tt + 1) * 128, :])
                ln(xt, xt)
                h1q = h1p.tile([128, D], bf16, tag="h1q", name="h1q")
                quant(xt, h1q, scaleA[tt])
                nc.sync.dma_start(out=h1t[:, :, tt, :], in_=h1q,
                                  transpose=True)
            nc.sync.dma_start(out=ag1_in[0:128, :],
                              in_=h1t[:].rearrange("p a b e -> p (a b e)"))
            for tt in range(4):
                nc.sync.dma_start(
                    out=ag1_in[128:129, tt * 256:(tt + 1) * 256]
                    .bitcast(f32), in_=scaleA[tt])
            nc.sync.dma_start(out=ag1_in[129:130, 0:12].bitcast(f32),
                              in_=wsp[0:1, 0:6])
            nc.gpsimd.collective_compute(
                "AllGather", Alu.bypass, replica_groups=r_all,
                ins=[ag1_in[:].opt()], outs=[ag1_out[:].opt()])

        # ---- P1: unpack scales/ws, quantize weights, issue weight AGs
        unpk = aw1.enter_context(tc.tile_pool(name="unpk", bufs=1))
        sc_raw = unpk.tile([1, 8192], bf16, name="sc_raw")
        for r in range(NCORES):
            nc.sync.dma_start(
                out=sc_raw[0:1, r * 1024:(r + 1) * 1024],
                in_=ag1_out[r * 130 + 128:r * 130 + 129, 0:1024])
        scaleA_row = sc_raw[:].bitcast(f32)          # [1, 4096]
        wsraw = unpk.tile([8, 12], bf16, name="wsraw")
        for r in range(NCORES):
            nc.sync.dma_start(
                out=wsraw[r:r + 1, :],
                in_=ag1_out[r * 130 + 129:r * 130 + 130, 0:12])
        wssum = unpk.tile([8, 6], f32, name="wssum")
        nc.gpsimd.partition_all_reduce(wssum[:], wsraw[:].bitcast(f32),
                                       channels=8, reduce_op=Red.add)
        rws = unpk.tile([1, 6], f32, name="rws")
        nc.vector.tensor_scalar(out=rws[0:1, 0:4], in0=wssum[0:1, 0:4],
                                scalar1=1.0 / (D * D), scalar2=1e-8,
                                op0=Alu.mult, op1=Alu.add)
        nc.vector.tensor_scalar(out=rws[0:1, 4:6], in0=wssum[0:1, 4:6],
                                scalar1=1.0 / (F * D), scalar2=1e-8,
                                op0=Alu.mult, op1=Alu.add)
        nc.vector.reciprocal(out=rws, in_=rws)
        rwsb = unpk.tile([128, 6], f32, name="rwsb")
        nc.gpsimd.partition_broadcast(rwsb[:], rws[0:1, :], channels=128)

        # per-batch cq columns (scaleA * 1/sqrt(HD)) for attention,
        # gathered from the DRAM scale rows (DRAM APs allow arbitrary
        # reshape; SBUF sources cannot synthesize partition steps)
        for b in range(2):
            for qi in range(16):
                r = 4 * b + qi // 4
                c0 = 2 * ((qi % 4) * 128)
                nc.sync.dma_start(
                    out=cqs[b][:, qi:qi + 1],
                    in_=ag1_out[r * 130 + 128:r * 130 + 129,
                                c0:c0 + 256].bitcast(f32))
            nc.vector.tensor_scalar(out=cqs[b], in0=cqs[b],
                                    scalar1=SMSCALE, scalar2=None,
                                    op0=Alu.mult)

        scale_bc = scbcp.tile([128, 4096], f32, name="scale_bc")
        nc.gpsimd.partition_broadcast(scale_bc[:], scaleA_row,
                                      channels=128)

        def wquant(src_ap, j, out_ap):
            q1 = perm.tile([128, 2048], f32, tag="wq1", name="wq1")
            nc.vector.tensor_scalar(out=q1, in0=src_ap,
                                    scalar1=rwsb[:, j:j + 1],
                                    scalar2=MAGIC, op0=Alu.mult,
                                    op1=Alu.add)
            nc.vector.tensor_scalar(out=q1, in0=q1, scalar1=MAGIC,
                                    scalar2=None, op0=Alu.subtract)
            nc.vector.tensor_scalar(out=out_ap, in0=q1, scalar1=-1.0,
                                    scalar2=1.0, op0=Alu.max, op1=Alu.min)

        # q/k/v ternary — SBUF resident (QKV stationary)
        wQ = {}
        for nm, d_, j in (("wq", wqT_d, 0), ("wk", wkT_d, 1),
                          ("wv", wvT_d, 2)):
            t = awp2.tile([128, 4096], bf16, name=f"tw_{nm}")
            for c in range(2):
                ft = wchkp.tile([128, 2048], f32, tag="wchk", name="wchk")
                nc.sync.dma_start(out=ft,
                                  in_=d_[:, c * 2048:(c + 1) * 2048])
                wquant(ft[:], j, t[:, c * 2048:(c + 1) * 2048])
            wQ[nm] = t
        # wo -> AG input (chunk c == a-block c); AG issued later
        for c in range(2):
            ft = wchkp.tile([128, 2048], f32, tag="wchk", name="wchk")
            nc.sync.dma_start(out=ft,
                              in_=woT_d[:, c * 2048:(c + 1) * 2048])
            qt = perm.tile([128, 2048], bf16, tag="wqb", name="wqb")
            wquant(ft[:], 3, qt[:])
            nc.sync.dma_start(out=agwo_in[c * 128:(c + 1) * 128, :],
                              in_=qt)
        # wu / wd AG inputs; 2048-chunk k -> (row, col) blocks of ag-in
        for d_, j, agin in ((wuT_d, 4, agwu_in), (wdT_d, 5, agwd_in)):
            for k in range(8):
                ft = wchkp.tile([128, 2048], f32, tag="wchk", name="wchk")
                nc.gpsimd.dma_start(out=ft,
                                    in_=d_[:, k * 2048:(k + 1) * 2048])
                qt = perm.tile([128, 2048], bf16, tag="wqb", name="wqb")
                wquant(ft[:], j, qt[:])
                nc.sync.dma_start(
                    out=agin[(k // 4) * 128:(k // 4) * 128 + 128,
                             (k % 4) * 2048:(k % 4 + 1) * 2048], in_=qt)
        aw1.close()

        a0 = ExitStack()
        a0p = a0.enter_context(tc.tile_pool(name="a0p", bufs=1))
        QT = [a0p.tile([128, 4096], f32, name=f"QT{i}") for i in range(2)]
        KS = [a0p.tile([128, 4096], f32, name=f"KS{i}") for i in range(2)]
        # V token-major, hi/lo split: [oc, b, kt, 128]
        vth = a0p.tile([128, 2, 2, 16, 128], bf16, name="vth")
        vtl = a0p.tile([128, 2, 2, 16, 128], bf16, name="vtl")

        # ---- P2: QKV ----
        with ExitStack() as s2:
            movp = s2.enter_context(tc.tile_pool(name="movp", bufs=3))
            scr = s2.enter_context(tc.tile_pool(name="scr", bufs=3))
            qkps = s2.enter_context(tc.tile_pool(name="qkps", bufs=6,
                                                 space="PSUM"))
            for mv in range(8):
                pss = [qkps.tile([128, TPC], f32, tag="ps", name="ps")
                       for _ in range(6)]
                for dt in range(16):
                    mt = movp.tile([128, TPC], bf16, tag="mov", name="mov")
                    nc.sync.dma_start(
                        out=mt,
                        in_=ag1_out[mv * 130:mv * 130 + 128,
                                    dt * 512:(dt + 1) * 512])
                    for im, nm in enumerate(("wq", "wk", "wv")):
                        wv_ = wQ[nm][:].rearrange("p (a o) -> p a o", o=256)
                        for oc in range(2):
                            nc.tensor.matmul(
                                pss[im * 2 + oc][:],
                                wv_[:, dt, oc * 128:(oc + 1) * 128],
                                mt[:], start=(dt == 0), stop=(dt == 15))
                sl = scale_bc[:, mv * 512:(mv + 1) * 512]
                dst = slice(mv * 512, (mv + 1) * 512)
                b_, q4 = mv // 4, mv % 4
                for oc in range(2):
                    nc.vector.tensor_copy(out=QT[oc][:, dst].bitcast(f32r),
                                          in_=pss[oc][:])
                    nc.vector.tensor_tensor(
                        out=KS[oc][:, dst].bitcast(f32r),
                        in0=pss[2 + oc][:], in1=sl, op=Alu.mult)
                    vt = scr.tile([128, TPC], f32, tag="vtmp", name="vtmp")
                    nc.vector.tensor_tensor(out=vt, in0=pss[4 + oc][:],
                                            in1=sl, op=Alu.mult)
                    vhi = scr.tile([128, TPC], bf16, tag="vhi", name="vhi")
                    nc.vector.tensor_copy(out=vhi, in_=vt)
                    vlo = scr.tile([128, TPC], bf16, tag="vlo", name="vlo")
                    nc.vector.tensor_tensor(out=vlo, in0=vt, in1=vhi[:],
                                            op=Alu.subtract)
                    nc.sync.dma_start(
                        out=vth[:, oc, b_, q4 * 4:q4 * 4 + 4, :],
                        in_=vhi, transpose=True)
                    nc.sync.dma_start(
                        out=vtl[:, oc, b_, q4 * 4:q4 * 4 + 4, :],
                        in_=vlo, transpose=True)
        # ---- P3: attention ----
        with ExitStack() as s3:
            attp = s3.enter_context(tc.tile_pool(name="attp", bufs=1))
            ptp = s3.enter_context(tc.tile_pool(name="ptp", bufs=2))
            pbp = s3.enter_context(tc.tile_pool(name="pbp", bufs=2))
            rbp = s3.enter_context(tc.tile_pool(name="rbp", bufs=2))
            owp = s3.enter_context(tc.tile_pool(name="owp", bufs=2))
            scps = s3.enter_context(tc.tile_pool(name="scps", bufs=1,
                                                 space="PSUM"))
            ovps = s3.enter_context(tc.tile_pool(name="ovps", bufs=2,
                                                 space="PSUM"))

            mask_sb = []
            for m in range(4):
                mt = attp.tile([128, 512], f32, name=f"msk{m}")
                nc.sync.dma_start(out=mt, in_=mask_d[m, :, :])
                mask_sb.append(mt)

            for b in range(2):
                for hl in range(2):
                    qv = QT[hl][:, b * 2048:(b + 1) * 2048].bitcast(f32r)
                    kv = KS[hl][:, b * 2048:(b + 1) * 2048].bitcast(f32r)
                    se = attp.tile([128, 16], f32, name=f"se{b}{hl}")
                    ps_o = None
                    for qi in range(16):
                        kvn = (qi + 1) * 128
                        nch = (kvn + 511) // 512
                        ps_s = scps.tile([128, 2048], f32, tag="sc",
                                         name="sc")
                        for ch in range(nch):
                            w = min(512, kvn - ch * 512)
                            nc.tensor.matmul(
                                ps_s[:, ch * 512:ch * 512 + w],
                                qv[:, qi * 128:(qi + 1) * 128],
                                kv[:, ch * 512:ch * 512 + w],
                                start=True, stop=True)
                        dw = (qi % 4 + 1) * 128
                        dbase = (qi // 4) * 512
                        nc.vector.tensor_add(
                            ps_s[:, dbase:dbase + dw],
                            ps_s[:, dbase:dbase + dw],
                            mask_sb[qi % 4][:, 0:dw])
                        m_ = stats.tile([128, 1], f32, tag="mrow",
                                        name="mrow")
                        nc.vector.tensor_reduce(out=m_,
                                                in_=ps_s[:, 0:kvn],
                                                axis=Ax.X, op=Alu.max)
                        bias = stats.tile([128, 1], f32, tag="bias",
                                          name="bias")
                        nc.vector.tensor_scalar(
                            out=bias, in0=m_,
                            scalar1=cqs[b][:, qi:qi + 1],
                            scalar2=-1.0, op0=Alu.mult, op1=Alu.mult)
                        probs = pbp.tile([128, 2048], bf16, tag="pb",
                                         name="pb")
                        nc.scalar.activation(
                            out=probs[:, 0:kvn], in_=ps_s[:, 0:kvn],
                            func=Act.Exp, bias=bias,
                            scale=cqs[b][:, qi:qi + 1],
                            accum_out=se[:, qi:qi + 1])
                        pt = ptp.tile([128, 2048], bf16, tag="pt",
                                      name="pt")
                        ptv = pt[:].rearrange("p (a e) -> p a e", e=128)
                        nc.sync.dma_start(out=ptv[:, 0:qi + 1, :],
                                          in_=probs[:, 0:kvn],
                                          transpose=True)
                        if qi % 4 == 0:
                            ps_o = ovps.tile([128, 512], f32, tag="ov",
                                             name="ov")
                        qo = (qi % 4) * 128
                        for kt in range(qi + 1):
                            nc.tensor.matmul(
                                ps_o[:, qo:qo + 128],
                                vth[:, hl, b, kt, :], ptv[:, kt, :],
                                start=(kt == 0), stop=False)
                        for kt in range(qi + 1):
                            nc.tensor.matmul(
                                ps_o[:, qo:qo + 128],
                                vtl[:, hl, b, kt, :], ptv[:, kt, :],
                                start=False, stop=(kt == qi))
                        if qi % 4 == 3:
                            gq = qi // 4
                            rse = stats.tile([128, 4], f32, tag="rse",
                                             name="rse")
                            nc.vector.reciprocal(
                                out=rse, in_=se[:, gq * 4:gq * 4 + 4])
                            rrow = rbp.tile([1, 512], f32, tag="rrow",
                                            name="rrow")
                            for k in range(4):
                                nc.sync.dma_start(
                                    out=rrow[0:1, k * 128:(k + 1) * 128],
                                    in_=rse[:, k:k + 1])
                            rd = rse_dram[(b * 2 + hl) % 2]
                            nc.sync.dma_start(out=rd[:], in_=rrow)
                            rbc = rbp.tile([128, 512], f32, tag="rbc",
                                           name="rbc")
                            nc.sync.dma_start(
                                out=rbc,
                                in_=rd[0:1, :].partition_broadcast(128))
                            ot = owp.tile([128, 512], f32, tag="ot",
                                          name="ot")
                            nc.vector.tensor_tensor(
                                out=ot, in0=ps_o[:], in1=rbc,
                                op=Alu.mult)
                            jg = b * 4 + gq
                            nc.sync.dma_start(
                                out=a2a_in[jg * 256 + hl * 128:
                                           jg * 256 + hl * 128 + 128, :],
                                in_=ot)
            nc.gpsimd.collective_compute(
                "AllGather", Alu.bypass, replica_groups=r_all,
                ins=[agwo_in[:].opt()], outs=[agwo_out[:].opt()])
            nc.gpsimd.collective_compute(
                "AllGather", Alu.bypass, replica_groups=r_all,
                ins=[agwu_in[:].opt()], outs=[agwu_out[:].opt()])
            nc.gpsimd.collective_compute(
                "AllGather", Alu.bypass, replica_groups=r_all,
                ins=[agwd_in[:].opt()], outs=[agwd_out[:].opt()])
            nc.gpsimd.collective_compute(
                "AllToAll", Alu.bypass, replica_groups=r_all,
                ins=[a2a_in[:].opt()], outs=[a2a_out[:].opt()])
        a0.close()
        aw2.close()

        pg2 = g.enter_context(tc.tile_pool(name="pg2", bufs=1))
        h2tT = pg2.tile([128, 16, 4, 128], bf16, name="h2tT")
        scc_bc = pg2.tile([128, 512], f32, name="scc_bc")

        # =============================================================
        # P4: o-quant + out-proj + residual (sequence-parallel)
        # =============================================================
        with ExitStack() as s4:
            x2pp = s4.enter_context(tc.tile_pool(name="x2pp", bufs=1))
            x2T = x2pp.tile([128, 16, 512], f32, name="x2T")
            with ExitStack() as s4i:
                otop = s4i.enter_context(tc.tile_pool(name="otop", bufs=3))
                oqp = s4i.enter_context(tc.tile_pool(name="oqp", bufs=1))
                wofp = s4i.enter_context(tc.tile_pool(name="wofp", bufs=1))
                prp = s4i.enter_context(tc.tile_pool(name="prp", bufs=2))
                wops = s4i.enter_context(tc.tile_pool(name="wops", bufs=4,
                                                      space="PSUM"))
                woF = wofp.tile([128, 16, 2048], bf16, name="woF")
                for r in range(NCORES):
                    nc.scalar.dma_start(
                        out=woF[:, r * 2:r * 2 + 2, :],
                        in_=agwo_out[r * 256:(r + 1) * 256, :]
                        .rearrange("(a p) o -> p a o", p=128))

                amax = x2pp.tile([1, 512], f32, name="oamax")
                for dt in range(16):
                    ot_ = otop.tile([128, 512], f32, tag="oto",
                                    name="oto")
                    nc.sync.dma_start(
                        out=ot_, in_=a2a_out[dt * 128:(dt + 1) * 128, :])
                    prt = prp.tile([128, 512], f32, tag="prt", name="prt")
                    nc.gpsimd.partition_all_reduce(prt[:], ot_[:],
                                                   channels=128,
                                                   reduce_op=Red.absmax)
                    if dt == 0:
                        nc.vector.tensor_copy(out=amax, in_=prt[0:1, :])
                    else:
                        nc.vector.tensor_tensor(out=amax, in0=amax,
                                                in1=prt[0:1, :],
                                                op=Alu.max)
                rqo_bc = x2pp.tile([128, 512], f32, name="rqo_bc")
                osc_bc = x2pp.tile([128, 512], f32, name="osc_bc")
                row_scales(x2pp, amax[0:1, :], rqo_bc, osc_bc)
                oqT = [oqp.tile([128, 512], bf16, name=f"oq{i}")
                       for i in range(16)]
                for dt in range(16):
                    ot_ = otop.tile([128, 512], f32, tag="oto",
                                    name="oto")
                    nc.sync.dma_start(
                        out=ot_, in_=a2a_out[dt * 128:(dt + 1) * 128, :])
                    tq = prp.tile([128, 512], f32, tag="tq", name="tq")
                    nc.vector.tensor_tensor(out=tq, in0=ot_[:],
                                            in1=rqo_bc, op=Alu.mult)
                    nc.vector.tensor_scalar(out=oqT[dt], in0=tq,
                                            scalar1=MAGIC, scalar2=MAGIC,
                                            op0=Alu.add, op1=Alu.subtract)
                for oc in range(16):
                    ps = wops.tile([128, 512], f32, tag="ps", name="ps")
                    for dt in range(16):
                        nc.tensor.matmul(
                            ps[:], woF[:, dt, oc * 128:(oc + 1) * 128],
                            oqT[dt][:], start=(dt == 0), stop=(dt == 15))
                    xr = prp.tile([128, 512], f32, tag="xr", name="xr")
                    nc.sync.dma_start(
                        out=xr, in_=xT_d[oc * 128:(oc + 1) * 128, :])
                    tp_ = prp.tile([128, 512], f32, tag="tp", name="tp")
                    nc.vector.tensor_tensor(out=tp_, in0=ps[:],
                                            in1=osc_bc, op=Alu.mult)
                    nc.vector.tensor_tensor(out=x2T[:, oc, :], in0=tp_,
                                            in1=xr, op=Alu.add)
                    nc.sync.dma_start(
                        out=x2t_dram[oc * 128:(oc + 1) * 128, :],
                        in_=x2T[:, oc, :])

            # ---- P5: x2 -> token-major (bf16 hi/lo), LN2, quant ----
            with ExitStack() as s5:
                x2p = s5.enter_context(tc.tile_pool(name="x2p", bufs=1))
                x2rp = s5.enter_context(tc.tile_pool(name="x2rp", bufs=2))
                x2hi = x2p.tile([128, 8192], bf16, name="x2hi")
                x2lo = x2p.tile([128, 8192], bf16, name="x2lo")
                x2flat = x2T[:].rearrange("p a e -> p (a e)")
                nc.vector.tensor_copy(out=x2hi, in_=x2flat)
                nc.vector.tensor_tensor(out=x2lo, in0=x2flat, in1=x2hi[:],
                                        op=Alu.subtract)
                xtth = x2p.tile([128, 64, 128], bf16, name="xtth")
                xttl = x2p.tile([128, 64, 128], bf16, name="xttl")
                nc.sync.dma_start(out=xtth, in_=x2hi[:], transpose=True)
                nc.sync.dma_start(out=xttl, in_=x2lo[:], transpose=True)
                vh4 = xtth[:].rearrange("p (a b) e -> p a b e", b=4)
                vl4 = xttl[:].rearrange("p (a b) e -> p a b e", b=4)
                scaleC = stats.tile([128, 4], f32, tag="scC", name="scC")
                scc_row = stats.tile([1, 512], f32, tag="sccr",
                                     name="sccr")
                for tt in range(4):
                    xt = x2rp.tile([128, 2048], f32, tag="x2tok",
                                   name="x2tok")
                    nc.vector.tensor_tensor(
                        out=xt, in0=vh4[:, :, tt, :], in1=vl4[:, :, tt, :],
                        op=Alu.add)
                    ln(xt, xt)
                    h2q = x2rp.tile([128, 2048], bf16, tag="h2q",
                                    name="h2q")
                    quant(xt, h2q, scaleC[:, tt:tt + 1])
                    nc.sync.dma_start(out=h2tT[:, :, tt, :], in_=h2q,
                                      transpose=True)
                    nc.sync.dma_start(
                        out=scc_row[0:1, tt * 128:(tt + 1) * 128],
                        in_=scaleC[:, tt:tt + 1])
                nc.gpsimd.partition_broadcast(
                    scc_bc[:], scc_row[0:1, :], channels=128)

        # =============================================================
        # P6: FFN up -> fp32 gelu (DRAM), u-quant; P7: FFN down
        # =============================================================
        with ExitStack() as s6:
            uqp = s6.enter_context(tc.tile_pool(name="uqp", bufs=1))
            ubcp = s6.enter_context(tc.tile_pool(name="ubcp", bufs=1))
            uamax = ubcp.tile([1, 512], f32, name="uamax")
            with ExitStack() as s6i:
                wup = s6i.enter_context(tc.tile_pool(name="wup", bufs=3))
                gtp = s6i.enter_context(tc.tile_pool(name="gtp", bufs=3))
                ffps = s6i.enter_context(tc.tile_pool(name="ffps", bufs=4,
                                                      space="PSUM"))
                h2f = h2tT[:].rearrange("p a b e -> p a (b e)")
                for fc in range(64):
                    r, lc = fc // 8, fc % 8
                    wub = wup.tile([128, 2048], bf16, tag="wu", name="wu")
                    nc.scalar.dma_start(
                        out=wub,
                        in_=agwu_out[r * 256 + (lc // 4) * 128:
                                     r * 256 + (lc // 4) * 128 + 128,
                                     (lc % 4) * 2048:(lc % 4 + 1) * 2048])
                    wuv = wub[:].rearrange("p (a e) -> p a e", e=128)
                    ps = ffps.tile([128, 512], f32, tag="ps", name="ps")
                    for dt in range(16):
                        nc.tensor.matmul(ps[:], wuv[:, dt, :],
                                         h2f[:, dt, :],
                                         start=(dt == 0), stop=(dt == 15))
                    tg = gtp.tile([128, 512], f32, tag="tg", name="tg")
                    nc.vector.tensor_tensor(out=tg, in0=ps[:], in1=scc_bc,
                                            op=Alu.mult)
                    ug = gtp.tile([128, 512], f32, tag="ug", name="ug")
                    nc.scalar.activation(out=ug, in_=tg, func=Act.Gelu,
                                         scale=1.0)
                    nc.sync.dma_start(
                        out=u_dram[fc * 128:(fc + 1) * 128, :], in_=ug)
                    prt = gtp.tile([128, 512], f32, tag="prt", name="prt")
                    nc.gpsimd.partition_all_reduce(prt[:], ug[:],
                                                   channels=128,
                                                   reduce_op=Red.absmax)
                    if fc == 0:
                        nc.vector.tensor_copy(out=uamax, in_=prt[0:1, :])
                    else:
                        nc.vector.tensor_tensor(out=uamax, in0=uamax,
                                                in1=prt[0:1, :],
                                                op=Alu.max)
            rqu_bc = ubcp.tile([128, 512], f32, name="rqu_bc")
            usc_bc = ubcp.tile([128, 512], f32, name="usc_bc")
            row_scales(ubcp, uamax[0:1, :], rqu_bc, usc_bc)
            Uq = [uqp.tile([128, 512], bf16, name=f"uq{i}")
                  for i in range(64)]
            with ExitStack() as s6q:
                ulp = s6q.enter_context(tc.tile_pool(name="ulp", bufs=3))
                for fc in range(64):
                    uf = ulp.tile([128, 512], f32, tag="uf", name="uf")
                    nc.sync.dma_start(
                        out=uf, in_=u_dram[fc * 128:(fc + 1) * 128, :])
                    tq = ulp.tile([128, 512], f32, tag="tq", name="tq")
                    nc.vector.tensor_tensor(out=tq, in0=uf[:], in1=rqu_bc,
                                            op=Alu.mult)
                    nc.vector.tensor_scalar(out=Uq[fc], in0=tq,
                                            scalar1=MAGIC, scalar2=MAGIC,
                                            op0=Alu.add, op1=Alu.subtract)

            # ---- P7: FFN down + residual -> out ----
            with ExitStack() as s7:
                wdp = s7.enter_context(tc.tile_pool(name="wdp", bufs=2))
                drp = s7.enter_context(tc.tile_pool(name="drp", bufs=2))
                dps = s7.enter_context(tc.tile_pool(name="dps", bufs=4,
                                                    space="PSUM"))
                agwd_v = agwd_out[:].rearrange("(r c p) f -> r c p f",
                                               c=2, p=128)
                for oc in range(16):
                    wdb = wdp.tile([128, 8192], bf16, tag="wd", name="wd")
                    nc.scalar.dma_start(
                        out=wdb[:].rearrange("p (r k) -> p r k", r=8),
                        in_=agwd_v[:, oc // 8, :,
                                   (oc % 8) * 1024:(oc % 8 + 1) * 1024]
                        .rearrange("r p k -> p r k"))
                    wdv = wdb[:].rearrange("p (a e) -> p a e", e=128)
                    ps = dps.tile([128, 512], f32, tag="ps", name="ps")
                    for fc in range(64):
                        nc.tensor.matmul(ps[:], wdv[:, fc, :], Uq[fc][:],
                                         start=(fc == 0), stop=(fc == 63))
                    x2r = drp.tile([128, 512], f32, tag="x2r", name="x2r")
                    nc.sync.dma_start(
                        out=x2r, in_=x2t_dram[oc * 128:(oc + 1) * 128, :])
                    t1 = drp.tile([128, 512], f32, tag="t1", name="t1")
                    nc.vector.tensor_tensor(out=t1, in0=ps[:], in1=usc_bc,
                                            op=Alu.mult)
                    t2 = drp.tile([128, 512], f32, tag="t2", name="t2")
                    nc.vector.tensor_tensor(out=t2, in0=t1, in1=x2r,
                                            op=Alu.add)
                    nc.sync.dma_start(
                        out=out_d[oc * 128:(oc + 1) * 128, :], in_=t2)

    nc.finalize()
    return nc


_CACHE = {}


def _pack_weights(wq, wk, wv, wo, wu, wd, r):
    # pure layout transforms (slice / transpose / reshape) — no arithmetic
    def pack_qkv(w):
        s = w[r * 256:(r + 1) * 256, :].T            # [2048, 256]
        return np.ascontiguousarray(
            s.reshape(16, 128, 256).transpose(1, 0, 2).reshape(128, 4096))

    def pack_wo(w):
        s = w.T[r * 256:(r + 1) * 256, :]            # [256, 2048]
        return np.ascontiguousarray(
            s.reshape(2, 128, 2048).transpose(1, 0, 2).reshape(128, 4096))

    def pack_wu(w):
        s = w[r * 1024:(r + 1) * 1024, :].T          # [2048, 1024]
        s = s.reshape(16, 128, 8, 128)               # [dblk, p, lc, fb]
        return np.ascontiguousarray(
            s.transpose(1, 2, 0, 3).reshape(128, 16384))

    def pack_wd(w):
        s = w.T[r * 1024:(r + 1) * 1024, :]          # [1024, 2048]
        s = s.reshape(8, 128, 16, 128)               # [fblk, p, oc, od]
        return np.ascontiguousarray(
            s.transpose(1, 2, 0, 3).reshape(128, 16384))

    return (pack_qkv(wq), pack_qkv(wk), pack_qkv(wv), pack_wo(wo),
            pack_wu(wu), pack_wd(wd))


def kernel(**inputs):
    x = np.asarray(inputs["x"], dtype=np.float32)
    wq = np.asarray(inputs["wq"], dtype=np.float32)
    wk = np.asarray(inputs["wk"], dtype=np.float32)
    wv = np.asarray(inputs["wv"], dtype=np.float32)
    wo = np.asarray(inputs["wo"], dtype=np.float32)
    wu = np.asarray(inputs["wu"], dtype=np.float32)
    wd = np.asarray(inputs["wd"], dtype=np.float32)

    if "nc" not in _CACHE:
        _CACHE["nc"] = build_graph()
    nc = _CACHE["nc"]

    mask = np.zeros((4, 128, 512), dtype=np.float32)
    jj = np.arange(512)[None, :]
    ii = np.arange(128)[:, None]
    for m in range(4):
        mask[m] = np.where(jj <= m * 128 + ii, 0.0, NEG)

    in_maps = []
    for r in range(NCORES):
        b, c = r // 4, r % 4
        xr = x[b, c * 512:(c + 1) * 512, :]
        wqp, wkp, wvp, wop, wup, wdp = _pack_weights(
            wq, wk, wv, wo, wu, wd, r)
        in_maps.append({
            "x": np.ascontiguousarray(xr),
            "xT": np.ascontiguousarray(xr.T),
            "wqT": wqp, "wkT": wkp, "wvT": wvp, "woT": wop,
            "wuT": wup, "wdT": wdp,
            "mask": mask,
        })

    from concourse import bass_utils
    res = bass_utils.run_bass_kernel_spmd(
        nc, in_maps, core_ids=list(range(NCORES)))
    _CACHE["last_result"] = res

    out = np.zeros((B, T, D), dtype=np.float32)
    for r in range(NCORES):
        b, c = r // 4, r % 4
        out[b, c * 512:(c + 1) * 512, :] = res.results[r]["out"].T
    return out


if __name__ == "__main__":
    nc = build_graph()
    n_inst = sum(len(bb.instructions) for bb in nc.main_func.blocks)
    print("graph built ok, instructions:", n_inst)


# revision 26
# speedup vs baseline: 1.0516x; 1.0516x over previous
"""BitNet transformer block on 8 Trainium2 NeuronCores — v2.

Hybrid tensor/sequence parallel chosen from HW-measured collective costs
(every collective costs ~80-200us regardless of size, so the design
minimizes collective COUNT; only shared-output 8-core AllGather and one
small AllToAll are used — no AllReduce/ReduceScatter):

  1. Each core LN1+quantizes its own 512 tokens; ONE AllGather ships the
     quantized tokens (bf16 ints, feature-major) with the fp32 token
     scales and the per-core |W| partial sums bit-packed into 2 extra
     bf16 rows.
  2. QKV + attention are head-parallel (2 heads x 2 batches per core),
     so K/V stay local.  Scores run in fp32r on integer Q/K (exact);
     V is scale-folded in fp32 and split hi/lo bf16 (exact to ~1e-5) so
     only the bf16 probs rounding remains (~4e-3).
  3. One 4MB fp32 AllToAll re-shards the attention output to
     token-sharded full-D, so the o-quant absmax is local.
  4. Out-proj and the whole FFN run sequence-parallel with FULL ternary
     weights delivered by three AllGathers (wo/wu/wd) issued early and
     overlapped with attention compute.  gelu stays fp32 (DRAM spill)
     and is re-quantized from fp32, eliminating bf16 requant error.

Weights arrive host-side pre-transposed/packed (layout only) so zero
weight transposes happen on device; activation transposes use multi-tile
XBAR transpose-DMA instructions (one per 128-row tile group).
"""

import numpy as np
from contextlib import ExitStack

B, T, D, H, HD, F = 2, 2048, 2048, 16, 128, 8192
NCORES = 8
TPC = 512            # tokens per core
MAGIC = 12582912.0   # 1.5*2**23: +M then -M rounds f32 to nearest-even int
INV127 = float(np.float32(1.0) / np.float32(127.0))
SMSCALE = float(np.float32(1.0) / np.sqrt(np.float32(HD)))
NEG = -1.0e30


def build_graph():
    import concourse.bass as bass
    import concourse.bacc as bacc
    import concourse.tile as tile
    from concourse import mybir, bass_isa

    f32 = mybir.dt.float32
    bf16 = mybir.dt.bfloat16
    f32r = mybir.dt.float32r
    Alu = mybir.AluOpType
    Act = mybir.ActivationFunctionType
    Ax = mybir.AxisListType
    Red = bass_isa.ReduceOp

    nc = bacc.Bacc("TRN2", target_bir_lowering=False, debug=False,
                   num_devices=NCORES)

    x_d = nc.dram_tensor("x", [TPC, D], f32, kind="ExternalInput").ap()
    xT_d = nc.dram_tensor("xT", [D, TPC], f32, kind="ExternalInput").ap()
    wqT_d = nc.dram_tensor("wqT", [128, 4096], f32, kind="ExternalInput").ap()
    wkT_d = nc.dram_tensor("wkT", [128, 4096], f32, kind="ExternalInput").ap()
    wvT_d = nc.dram_tensor("wvT", [128, 4096], f32, kind="ExternalInput").ap()
    woT_d = nc.dram_tensor("woT", [128, 4096], f32, kind="ExternalInput").ap()
    wuT_d = nc.dram_tensor("wuT", [128, 16384], f32,
                           kind="ExternalInput").ap()
    wdT_d = nc.dram_tensor("wdT", [128, 16384], f32,
                           kind="ExternalInput").ap()
    mask_d = nc.dram_tensor("mask", [4, 128, 512], f32,
                            kind="ExternalInput").ap()
    out_d = nc.dram_tensor("out", [D, TPC], f32, kind="ExternalOutput").ap()

    r_all = [list(range(NCORES))]

    with tile.TileContext(nc) as tc, ExitStack() as g:
        dram = g.enter_context(tc.tile_pool(name="dram", bufs=1,
                                            space="DRAM"))
        p0 = g.enter_context(tc.tile_pool(name="p0", bufs=1))
        stats = g.enter_context(tc.tile_pool(name="stats", bufs=2))
        perm = g.enter_context(tc.tile_pool(name="perm", bufs=2))

        # ---- DRAM buffers ----
        ag1_in = dram.tile([130, 8192], bf16, name="ag1_in")
        ag1_out = dram.tile([130 * NCORES, 8192], bf16, name="ag1_out",
                            addr_space="Shared")
        agwo_in = dram.tile([256, 2048], bf16, name="agwo_in")
        agwo_out = dram.tile([2048, 2048], bf16, name="agwo_out",
                             addr_space="Shared")
        agwu_in = dram.tile([256, 8192], bf16, name="agwu_in")
        agwu_out = dram.tile([2048, 8192], bf16, name="agwu_out",
                             addr_space="Shared")
        agwd_in = dram.tile([256, 8192], bf16, name="agwd_in")
        agwd_out = dram.tile([2048, 8192], bf16, name="agwd_out",
                             addr_space="Shared")
        a2a_in = dram.tile([2048, TPC], f32, name="a2a_in")
        a2a_out = dram.tile([2048, TPC], f32, name="a2a_out")
        x2t_dram = dram.tile([D, TPC], f32, name="x2t_dram")
        rse_dram = [dram.tile([1, 512], f32, name=f"rse_dram{i}")
                    for i in range(2)]
        u_dram = dram.tile([F, TPC], f32, name="u_dram")

        eps_t = p0.tile([128, 1], f32, name="eps")
        nc.vector.memset(eps_t, 1.0e-5)
        scaleA = [p0.tile([128, 1], f32, name=f"sA{i}") for i in range(4)]

        def ln(src, dst):
            st = stats.tile([128, 4, 6], f32, tag="bn", name="bn")
            for sg in range(4):
                nc.vector.bn_stats(out=st[:, sg, :],
                                   in_=src[:, sg * 512:(sg + 1) * 512])
            mv = stats.tile([128, 2], f32, tag="mv", name="mv")
            nc.vector.bn_aggr(out=mv, in_=st)
            sq = stats.tile([128, 1], f32, tag="sq", name="sq")
            nc.scalar.activation(out=sq, in_=mv[:, 1:2], func=Act.Sqrt,
                                 bias=eps_t, scale=1.0)
            rstd = stats.tile([128, 1], f32, tag="rstd", name="rstd")
            nc.vector.reciprocal(out=rstd, in_=sq)
            nc.vector.tensor_scalar(out=dst, in0=src, scalar1=mv[:, 0:1],
                                    scalar2=rstd, op0=Alu.subtract,
                                    op1=Alu.mult)

        def quant(h, xq_out, scale_out):
            # NOTE: destroys h (rounding pass is in-place)
            amax = stats.tile([128, 1], f32, tag="amax", name="amax")
            nc.vector.tensor_reduce(out=amax, in_=h, axis=Ax.X, op=Alu.max,
                                    apply_absolute_value=True)
            nc.vector.tensor_scalar(out=scale_out, in0=amax, scalar1=INV127,
                                    scalar2=None, op0=Alu.mult)
            rq = stats.tile([128, 1], f32, tag="rq", name="rq")
            nc.vector.tensor_scalar(out=rq, in0=scale_out, scalar1=1e-8,
                                    scalar2=None, op0=Alu.add)
            nc.vector.reciprocal(out=rq, in_=rq)
            nc.vector.tensor_scalar(out=h, in0=h, scalar1=rq, scalar2=MAGIC,
                                    op0=Alu.mult, op1=Alu.add)
            nc.vector.tensor_scalar(out=xq_out, in0=h, scalar1=MAGIC,
                                    scalar2=None, op0=Alu.subtract)

        def row_scales(pool, row_in, rq_bc, sc_bc):
            """amax row [1,512] -> scale row + reciprocal, broadcast."""
            sc_row = pool.tile([1, 512], f32, tag="scrow", name="scrow")
            nc.vector.tensor_scalar(out=sc_row, in0=row_in, scalar1=INV127,
                                    scalar2=None, op0=Alu.mult)
            rq_row = pool.tile([1, 512], f32, tag="rqrow", name="rqrow")
            nc.vector.tensor_scalar(out=rq_row, in0=sc_row, scalar1=1e-8,
                                    scalar2=None, op0=Alu.add)
            nc.vector.reciprocal(out=rq_row, in_=rq_row)
            nc.gpsimd.partition_broadcast(rq_bc[:], rq_row[0:1, :],
                                          channels=128)
            nc.gpsimd.partition_broadcast(sc_bc[:], sc_row[0:1, :],
                                          channels=128)

        # scope nesting (LIFO): aw2 [P0..P3] > {aw1 [P0..P1], a0 [P2..P3]}
        cqs = [perm.tile([128, 16], f32, name=f"cqs{b}") for b in range(2)]
        aw2 = ExitStack()
        awp2 = aw2.enter_context(tc.tile_pool(name="awp2", bufs=1))
        scbcp = aw2.enter_context(tc.tile_pool(name="scbcp", bufs=1))
        aw1 = ExitStack()
        awp = aw1.enter_context(tc.tile_pool(name="awp", bufs=1))
        wchkp = aw1.enter_context(tc.tile_pool(name="wchk", bufs=2))

        # ---- P0: |W| stats (streamed in 1MB chunks) + LN1/quant + AG1
        srcs = [("wq", wqT_d, 2), ("wk", wkT_d, 2), ("wv", wvT_d, 2),
                ("wo", woT_d, 2), ("wu", wuT_d, 8), ("wd", wdT_d, 8)]
        pp = awp.tile([128, 24], f32, name="wpart")
        col = 0
        for nm, d_, nch_ in srcs:
            for c in range(nch_):
                t = wchkp.tile([128, 2048], f32, tag="wchk", name="wchk")
                nc.gpsimd.dma_start(out=t,
                                    in_=d_[:, c * 2048:(c + 1) * 2048])
                nc.vector.tensor_reduce(out=pp[:, col:col + 1], in_=t,
                                        axis=Ax.X, op=Alu.add,
                                        apply_absolute_value=True)
                col += 1
        ppr = awp.tile([128, 24], f32, name="wpartr")
        nc.gpsimd.partition_all_reduce(ppr[:], pp[:], channels=128,
                                       reduce_op=Red.add)
        wsp = awp.tile([1, 8], f32, name="wsp")
        col = 0
        for j, (nm, d_, nch_) in enumerate(srcs):
            nc.vector.tensor_reduce(out=wsp[0:1, j:j + 1],
                                    in_=ppr[0:1, col:col + nch_],
                                    axis=Ax.X, op=Alu.add)
            col += nch_

        with ExitStack() as s0:
            xp = s0.enter_context(tc.tile_pool(name="xp", bufs=2))
            h1p = s0.enter_context(tc.tile_pool(name="h1p", bufs=2))
            h1tp = s0.enter_context(tc.tile_pool(name="h1tp", bufs=1))
            h1t = h1tp.tile([128, 16, 4, 128], bf16, name="h1t")
            for tt in range(4):
                xt = xp.tile([128, D], f32, tag="xt", name="xt")
                nc.sync.dma_start(out=xt,
                                  in_=x_d[tt * 128:(tt + 1) * 128, :])
                ln(xt, xt)
                h1q = h1p.tile([128, D], bf16, tag="h1q", name="h1q")
                quant(xt, h1q, scaleA[tt])
                nc.sync.dma_start(out=h1t[:, :, tt, :], in_=h1q,
                                  transpose=True)
            nc.sync.dma_start(out=ag1_in[0:128, :],
                              in_=h1t[:].rearrange("p a b e -> p (a b e)"))
            for tt in range(4):
                nc.sync.dma_start(
                    out=ag1_in[128:129, tt * 256:(tt + 1) * 256]
                    .bitcast(f32), in_=scaleA[tt])
            nc.sync.dma_start(out=ag1_in[129:130, 0:12].bitcast(f32),
                              in_=wsp[0:1, 0:6])
            nc.gpsimd.collective_compute(
                "AllGather", Alu.bypass, replica_groups=r_all,
                ins=[ag1_in[:].opt()], outs=[ag1_out[:].opt()])

        # ---- P1: unpack scales/ws, quantize weights, issue weight AGs
        unpk = aw1.enter_context(tc.tile_pool(name="unpk", bufs=1))
        sc_raw = unpk.tile([1, 8192], bf16, name="sc_raw")
        for r in range(NCORES):
            nc.sync.dma_start(
                out=sc_raw[0:1, r * 1024:(r + 1) * 1024],
                in_=ag1_out[r * 130 + 128:r * 130 + 129, 0:1024])
        scaleA_row = sc_raw[:].bitcast(f32)          # [1, 4096]
        wsraw = unpk.tile([8, 12], bf16, name="wsraw")
        for r in range(NCORES):
            nc.sync.dma_start(
                out=wsraw[r:r + 1, :],
                in_=ag1_out[r * 130 + 129:r * 130 + 130, 0:12])
        wssum = unpk.tile([8, 6], f32, name="wssum")
        nc.gpsimd.partition_all_reduce(wssum[:], wsraw[:].bitcast(f32),
                                       channels=8, reduce_op=Red.add)
        rws = unpk.tile([1, 6], f32, name="rws")
        nc.vector.tensor_scalar(out=rws[0:1, 0:4], in0=wssum[0:1, 0:4],
                                scalar1=1.0 / (D * D), scalar2=1e-8,
                                op0=Alu.mult, op1=Alu.add)
        nc.vector.tensor_scalar(out=rws[0:1, 4:6], in0=wssum[0:1, 4:6],
                                scalar1=1.0 / (F * D), scalar2=1e-8,
                                op0=Alu.mult, op1=Alu.add)
        nc.vector.reciprocal(out=rws, in_=rws)
        rwsb = unpk.tile([128, 6], f32, name="rwsb")
        nc.gpsimd.partition_broadcast(rwsb[:], rws[0:1, :], channels=128)

        # per-batch cq columns (scaleA * 1/sqrt(HD)) for attention,
        # gathered from the DRAM scale rows (DRAM APs allow arbitrary
        # reshape; SBUF sources cannot synthesize partition steps)
        for b in range(2):
            for qi in range(16):
                r = 4 * b + qi // 4
                c0 = 2 * ((qi % 4) * 128)
                nc.sync.dma_start(
                    out=cqs[b][:, qi:qi + 1],
                    in_=ag1_out[r * 130 + 128:r * 130 + 129,
                                c0:c0 + 256].bitcast(f32))
            nc.vector.tensor_scalar(out=cqs[b], in0=cqs[b],
                                    scalar1=SMSCALE, scalar2=None,
                                    op0=Alu.mult)

        scale_bc = scbcp.tile([128, 4096], f32, name="scale_bc")
        nc.gpsimd.partition_broadcast(scale_bc[:], scaleA_row,
                                      channels=128)

        def wquant(src_ap, j, out_ap):
            q1 = perm.tile([128, 2048], f32, tag="wq1", name="wq1")
            nc.vector.tensor_scalar(out=q1, in0=src_ap,
                                    scalar1=rwsb[:, j:j + 1],
                                    scalar2=MAGIC, op0=Alu.mult,
                                    op1=Alu.add)
            nc.vector.tensor_scalar(out=q1, in0=q1, scalar1=MAGIC,
                                    scalar2=None, op0=Alu.subtract)
            nc.vector.tensor_scalar(out=out_ap, in0=q1, scalar1=-1.0,
                                    scalar2=1.0, op0=Alu.max, op1=Alu.min)

        # q/k/v ternary — SBUF resident (QKV stationary)
        wQ = {}
        for nm, d_, j in (("wq", wqT_d, 0), ("wk", wkT_d, 1),
                          ("wv", wvT_d, 2)):
            t = awp2.tile([128, 4096], bf16, name=f"tw_{nm}")
            for c in range(2):
                ft = wchkp.tile([128, 2048], f32, tag="wchk", name="wchk")
                nc.sync.dma_start(out=ft,
                                  in_=d_[:, c * 2048:(c + 1) * 2048])
                wquant(ft[:], j, t[:, c * 2048:(c + 1) * 2048])
            wQ[nm] = t
        # wo -> AG input (chunk c == a-block c); AG issued later
        for c in range(2):
            ft = wchkp.tile([128, 2048], f32, tag="wchk", name="wchk")
            nc.sync.dma_start(out=ft,
                              in_=woT_d[:, c * 2048:(c + 1) * 2048])
            qt = perm.tile([128, 2048], bf16, tag="wqb", name="wqb")
            wquant(ft[:], 3, qt[:])
            nc.sync.dma_start(out=agwo_in[c * 128:(c + 1) * 128, :],
                              in_=qt)
        # wu / wd AG inputs; 2048-chunk k -> (row, col) blocks of ag-in
        for d_, j, agin in ((wuT_d, 4, agwu_in), (wdT_d, 5, agwd_in)):
            for k in range(8):
                ft = wchkp.tile([128, 2048], f32, tag="wchk", name="wchk")
                nc.gpsimd.dma_start(out=ft,
                                    in_=d_[:, k * 2048:(k + 1) * 2048])
                qt = perm.tile([128, 2048], bf16, tag="wqb", name="wqb")
                wquant(ft[:], j, qt[:])
                nc.sync.dma_start(
                    out=agin[(k // 4) * 128:(k // 4) * 128 + 128,
                             (k % 4) * 2048:(k % 4 + 1) * 2048], in_=qt)
        aw1.close()

        a0 = ExitStack()
        a0p = a0.enter_context(tc.tile_pool(name="a0p", bufs=1))
        QT = [a0p.tile([128, 4096], f32, name=f"QT{i}") for i in range(2)]
        KS = [a0p.tile([128, 4096], f32, name=f"KS{i}") for i in range(2)]
        # V token-major, hi/lo split: [oc, b, kt, 128]
        vth = a0p.tile([128, 2, 2, 16, 128], bf16, name="vth")
        vtl = a0p.tile([128, 2, 2, 16, 128], bf16, name="vtl")

        # ---- P2: QKV ----
        with ExitStack() as s2:
            movp = s2.enter_context(tc.tile_pool(name="movp", bufs=3))
            scr = s2.enter_context(tc.tile_pool(name="scr", bufs=3))
            vhp = s2.enter_context(tc.tile_pool(name="vhp", bufs=1))
            qkps = s2.enter_context(tc.tile_pool(name="qkps", bufs=6,
                                                 space="PSUM"))
            VH = [vhp.tile([128, 4096], bf16, name=f"VH{i}")
                  for i in range(2)]
            VL = [vhp.tile([128, 4096], bf16, name=f"VL{i}")
                  for i in range(2)]
            for mv in range(8):
                pss = [qkps.tile([128, TPC], f32, tag="ps", name="ps")
                       for _ in range(6)]
                for dt in range(16):
                    mt = movp.tile([128, TPC], bf16, tag="mov", name="mov")
                    nc.sync.dma_start(
                        out=mt,
                        in_=ag1_out[mv * 130:mv * 130 + 128,
                                    dt * 512:(dt + 1) * 512])
                    for im, nm in enumerate(("wq", "wk", "wv")):
                        wv_ = wQ[nm][:].rearrange("p (a o) -> p a o", o=256)
                        for oc in range(2):
                            nc.tensor.matmul(
                                pss[im * 2 + oc][:],
                                wv_[:, dt, oc * 128:(oc + 1) * 128],
                                mt[:], start=(dt == 0), stop=(dt == 15))
                sl = scale_bc[:, mv * 512:(mv + 1) * 512]
                dst = slice(mv * 512, (mv + 1) * 512)
                b_, q4 = mv // 4, mv % 4
                for oc in range(2):
                    nc.vector.tensor_copy(out=QT[oc][:, dst].bitcast(f32r),
                                          in_=pss[oc][:])
                    nc.vector.tensor_tensor(
                        out=KS[oc][:, dst].bitcast(f32r),
                        in0=pss[2 + oc][:], in1=sl, op=Alu.mult)
                    vt = scr.tile([128, TPC], f32, tag="vtmp", name="vtmp")
                    nc.vector.tensor_tensor(out=vt, in0=pss[4 + oc][:],
                                            in1=sl, op=Alu.mult)
                    nc.vector.tensor_copy(out=VH[oc][:, dst], in_=vt)
                    nc.vector.tensor_tensor(out=VL[oc][:, dst], in0=vt,
                                            in1=VH[oc][:, dst],
                                            op=Alu.subtract)
            # all QKV DMAs are queued; V transposes go last so the
            # collective-completion barrier they attract cannot stall
            # the QKV moving-load stream
            for oc in range(2):
                for b_ in range(2):
                    csl = slice(b_ * 2048, (b_ + 1) * 2048)
                    nc.sync.dma_start(out=vth[:, oc, b_, :, :],
                                      in_=VH[oc][:, csl], transpose=True)
                    nc.sync.dma_start(out=vtl[:, oc, b_, :, :],
                                      in_=VL[oc][:, csl], transpose=True)
        # ---- P3: attention ----
        with ExitStack() as s3:
            attp = s3.enter_context(tc.tile_pool(name="attp", bufs=1))
            ptp = s3.enter_context(tc.tile_pool(name="ptp", bufs=2))
            pbp = s3.enter_context(tc.tile_pool(name="pbp", bufs=2))
            rbp = s3.enter_context(tc.tile_pool(name="rbp", bufs=2))
            owp = s3.enter_context(tc.tile_pool(name="owp", bufs=2))
            scps = s3.enter_context(tc.tile_pool(name="scps", bufs=1,
                                                 space="PSUM"))
            ovps = s3.enter_context(tc.tile_pool(name="ovps", bufs=2,
                                                 space="PSUM"))

            mask_sb = []
            for m in range(4):
                mt = attp.tile([128, 512], f32, name=f"msk{m}")
                nc.sync.dma_start(out=mt, in_=mask_d[m, :, :])
                mask_sb.append(mt)

            for b in range(2):
                for hl in range(2):
                    qv = QT[hl][:, b * 2048:(b + 1) * 2048].bitcast(f32r)
                    kv = KS[hl][:, b * 2048:(b + 1) * 2048].bitcast(f32r)
                    se = attp.tile([128, 16], f32, name=f"se{b}{hl}")
                    ps_o = None
                    for qi in range(16):
                        kvn = (qi + 1) * 128
                        nch = (kvn + 511) // 512
                        ps_s = scps.tile([128, 2048], f32, tag="sc",
                                         name="sc")
                        for ch in range(nch):
                            w = min(512, kvn - ch * 512)
                            nc.tensor.matmul(
                                ps_s[:, ch * 512:ch * 512 + w],
                                qv[:, qi * 128:(qi + 1) * 128],
                                kv[:, ch * 512:ch * 512 + w],
                                start=True, stop=True)
                        dw = (qi % 4 + 1) * 128
                        dbase = (qi // 4) * 512
                        nc.vector.tensor_add(
                            ps_s[:, dbase:dbase + dw],
                            ps_s[:, dbase:dbase + dw],
                            mask_sb[qi % 4][:, 0:dw])
                        m_ = stats.tile([128, 1], f32, tag="mrow",
                                        name="mrow")
                        nc.vector.tensor_reduce(out=m_,
                                                in_=ps_s[:, 0:kvn],
                                                axis=Ax.X, op=Alu.max)
                        bias = stats.tile([128, 1], f32, tag="bias",
                                          name="bias")
                        nc.vector.tensor_scalar(
                            out=bias, in0=m_,
                            scalar1=cqs[b][:, qi:qi + 1],
                            scalar2=-1.0, op0=Alu.mult, op1=Alu.mult)
                        probs = pbp.tile([128, 2048], bf16, tag="pb",
                                         name="pb")
                        nc.scalar.activation(
                            out=probs[:, 0:kvn], in_=ps_s[:, 0:kvn],
                            func=Act.Exp, bias=bias,
                            scale=cqs[b][:, qi:qi + 1],
                            accum_out=se[:, qi:qi + 1])
                        pt = ptp.tile([128, 2048], bf16, tag="pt",
                                      name="pt")
                        ptv = pt[:].rearrange("p (a e) -> p a e", e=128)
                        nc.sync.dma_start(out=ptv[:, 0:qi + 1, :],
                                          in_=probs[:, 0:kvn],
                                          transpose=True)
                        if qi % 4 == 0:
                            ps_o = ovps.tile([128, 512], f32, tag="ov",
                                             name="ov")
                        qo = (qi % 4) * 128
                        for kt in range(qi + 1):
                            nc.tensor.matmul(
                                ps_o[:, qo:qo + 128],
                                vth[:, hl, b, kt, :], ptv[:, kt, :],
                                start=(kt == 0), stop=False)
                        for kt in range(qi + 1):
                            nc.tensor.matmul(
                                ps_o[:, qo:qo + 128],
                                vtl[:, hl, b, kt, :], ptv[:, kt, :],
                                start=False, stop=(kt == qi))
                        if qi % 4 == 3:
                            gq = qi // 4
                            rse = stats.tile([128, 4], f32, tag="rse",
                                             name="rse")
                            nc.vector.reciprocal(
                                out=rse, in_=se[:, gq * 4:gq * 4 + 4])
                            rrow = rbp.tile([1, 512], f32, tag="rrow",
                                            name="rrow")
                            for k in range(4):
                                nc.sync.dma_start(
                                    out=rrow[0:1, k * 128:(k + 1) * 128],
                                    in_=rse[:, k:k + 1])
                            rd = rse_dram[(b * 2 + hl) % 2]
                            nc.sync.dma_start(out=rd[:], in_=rrow)
                            rbc = rbp.tile([128, 512], f32, tag="rbc",
                                           name="rbc")
                            nc.sync.dma_start(
                                out=rbc,
                                in_=rd[0:1, :].partition_broadcast(128))
                            ot = owp.tile([128, 512], f32, tag="ot",
                                          name="ot")
                            nc.vector.tensor_tensor(
                                out=ot, in0=ps_o[:], in1=rbc,
                                op=Alu.mult)
                            jg = b * 4 + gq
                            nc.sync.dma_start(
                                out=a2a_in[jg * 256 + hl * 128:
                                           jg * 256 + hl * 128 + 128, :],
                                in_=ot)
            nc.gpsimd.collective_compute(
                "AllGather", Alu.bypass, replica_groups=r_all,
                ins=[agwo_in[:].opt()], outs=[agwo_out[:].opt()])
            nc.gpsimd.collective_compute(
                "AllGather", Alu.bypass, replica_groups=r_all,
                ins=[agwu_in[:].opt()], outs=[agwu_out[:].opt()])
            nc.gpsimd.collective_compute(
                "AllGather", Alu.bypass, replica_groups=r_all,
                ins=[agwd_in[:].opt()], outs=[agwd_out[:].opt()])
            nc.gpsimd.collective_compute(
                "AllToAll", Alu.bypass, replica_groups=r_all,
                ins=[a2a_in[:].opt()], outs=[a2a_out[:].opt()])
        a0.close()
        aw2.close()

        pg2 = g.enter_context(tc.tile_pool(name="pg2", bufs=1))
        h2tT = pg2.tile([128, 16, 4, 128], bf16, name="h2tT")
        scc_bc = pg2.tile([128, 512], f32, name="scc_bc")

        # =============================================================
        # P4: o-quant + out-proj + residual (sequence-parallel)
        # =============================================================
        with ExitStack() as s4:
            x2pp = s4.enter_context(tc.tile_pool(name="x2pp", bufs=1))
            x2T = x2pp.tile([128, 16, 512], f32, name="x2T")
            with ExitStack() as s4i:
                otop = s4i.enter_context(tc.tile_pool(name="otop", bufs=3))
                oqp = s4i.enter_context(tc.tile_pool(name="oqp", bufs=1))
                wofp = s4i.enter_context(tc.tile_pool(name="wofp", bufs=1))
                prp = s4i.enter_context(tc.tile_pool(name="prp", bufs=2))
                wops = s4i.enter_context(tc.tile_pool(name="wops", bufs=4,
                                                      space="PSUM"))
                woF = wofp.tile([128, 16, 2048], bf16, name="woF")
                for r in range(NCORES):
                    nc.scalar.dma_start(
                        out=woF[:, r * 2:r * 2 + 2, :],
                        in_=agwo_out[r * 256:(r + 1) * 256, :]
                        .rearrange("(a p) o -> p a o", p=128))

                amax = x2pp.tile([1, 512], f32, name="oamax")
                for dt in range(16):
                    ot_ = otop.tile([128, 512], f32, tag="oto",
                                    name="oto")
                    nc.sync.dma_start(
                        out=ot_, in_=a2a_out[dt * 128:(dt + 1) * 128, :])
                    prt = prp.tile([128, 512], f32, tag="prt", name="prt")
                    nc.gpsimd.partition_all_reduce(prt[:], ot_[:],
                                                   channels=128,
                                                   reduce_op=Red.absmax)
                    if dt == 0:
                        nc.vector.tensor_copy(out=amax, in_=prt[0:1, :])
                    else:
                        nc.vector.tensor_tensor(out=amax, in0=amax,
                                                in1=prt[0:1, :],
                                                op=Alu.max)
                rqo_bc = x2pp.tile([128, 512], f32, name="rqo_bc")
                osc_bc = x2pp.tile([128, 512], f32, name="osc_bc")
                row_scales(x2pp, amax[0:1, :], rqo_bc, osc_bc)
                oqT = [oqp.tile([128, 512], bf16, name=f"oq{i}")
                       for i in range(16)]
                for dt in range(16):
                    ot_ = otop.tile([128, 512], f32, tag="oto",
                                    name="oto")
                    nc.sync.dma_start(
                        out=ot_, in_=a2a_out[dt * 128:(dt + 1) * 128, :])
                    tq = prp.tile([128, 512], f32, tag="tq", name="tq")
                    nc.vector.tensor_tensor(out=tq, in0=ot_[:],
                                            in1=rqo_bc, op=Alu.mult)
                    nc.vector.tensor_scalar(out=oqT[dt], in0=tq,
                                            scalar1=MAGIC, scalar2=MAGIC,
                                            op0=Alu.add, op1=Alu.subtract)
                for oc in range(16):
                    ps = wops.tile([128, 512], f32, tag="ps", name="ps")
                    for dt in range(16):
                        nc.tensor.matmul(
                            ps[:], woF[:, dt, oc * 128:(oc + 1) * 128],
                            oqT[dt][:], start=(dt == 0), stop=(dt == 15))
                    xr = prp.tile([128, 512], f32, tag="xr", name="xr")
                    nc.sync.dma_start(
                        out=xr, in_=xT_d[oc * 128:(oc + 1) * 128, :])
                    tp_ = prp.tile([128, 512], f32, tag="tp", name="tp")
                    nc.vector.tensor_tensor(out=tp_, in0=ps[:],
                                            in1=osc_bc, op=Alu.mult)
                    nc.vector.tensor_tensor(out=x2T[:, oc, :], in0=tp_,
                                            in1=xr, op=Alu.add)
                    nc.sync.dma_start(
                        out=x2t_dram[oc * 128:(oc + 1) * 128, :],
                        in_=x2T[:, oc, :])

            # ---- P5: x2 -> token-major (bf16 hi/lo), LN2, quant ----
            with ExitStack() as s5:
                x2p = s5.enter_context(tc.tile_pool(name="x2p", bufs=1))
                x2rp = s5.enter_context(tc.tile_pool(name="x2rp", bufs=2))
                x2hi = x2p.tile([128, 8192], bf16, name="x2hi")
                x2lo = x2p.tile([128, 8192], bf16, name="x2lo")
                x2flat = x2T[:].rearrange("p a e -> p (a e)")
                nc.vector.tensor_copy(out=x2hi, in_=x2flat)
                nc.vector.tensor_tensor(out=x2lo, in0=x2flat, in1=x2hi[:],
                                        op=Alu.subtract)
                xtth = x2p.tile([128, 64, 128], bf16, name="xtth")
                xttl = x2p.tile([128, 64, 128], bf16, name="xttl")
                nc.sync.dma_start(out=xtth, in_=x2hi[:], transpose=True)
                nc.sync.dma_start(out=xttl, in_=x2lo[:], transpose=True)
                vh4 = xtth[:].rearrange("p (a b) e -> p a b e", b=4)
                vl4 = xttl[:].rearrange("p (a b) e -> p a b e", b=4)
                scaleC = stats.tile([128, 4], f32, tag="scC", name="scC")
                scc_row = stats.tile([1, 512], f32, tag="sccr",
                                     name="sccr")
                for tt in range(4):
                    xt = x2rp.tile([128, 2048], f32, tag="x2tok",
                                   name="x2tok")
                    nc.vector.tensor_tensor(
                        out=xt, in0=vh4[:, :, tt, :], in1=vl4[:, :, tt, :],
                        op=Alu.add)
                    ln(xt, xt)
                    h2q = x2rp.tile([128, 2048], bf16, tag="h2q",
                                    name="h2q")
                    quant(xt, h2q, scaleC[:, tt:tt + 1])
                    nc.sync.dma_start(out=h2tT[:, :, tt, :], in_=h2q,
                                      transpose=True)
                    nc.sync.dma_start(
                        out=scc_row[0:1, tt * 128:(tt + 1) * 128],
                        in_=scaleC[:, tt:tt + 1])
                nc.gpsimd.partition_broadcast(
                    scc_bc[:], scc_row[0:1, :], channels=128)

        # =============================================================
        # P6: FFN up -> fp32 gelu (DRAM), u-quant; P7: FFN down
        # =============================================================
        with ExitStack() as s6:
            uqp = s6.enter_context(tc.tile_pool(name="uqp", bufs=1))
            ubcp = s6.enter_context(tc.tile_pool(name="ubcp", bufs=1))
            uamax = ubcp.tile([1, 512], f32, name="uamax")
            with ExitStack() as s6i:
                wup = s6i.enter_context(tc.tile_pool(name="wup", bufs=3))
                gtp = s6i.enter_context(tc.tile_pool(name="gtp", bufs=3))
                ffps = s6i.enter_context(tc.tile_pool(name="ffps", bufs=4,
                                                      space="PSUM"))
                h2f = h2tT[:].rearrange("p a b e -> p a (b e)")
                for fc in range(64):
                    r, lc = fc // 8, fc % 8
                    wub = wup.tile([128, 2048], bf16, tag="wu", name="wu")
                    nc.scalar.dma_start(
                        out=wub,
                        in_=agwu_out[r * 256 + (lc // 4) * 128:
                                     r * 256 + (lc // 4) * 128 + 128,
                                     (lc % 4) * 2048:(lc % 4 + 1) * 2048])
                    wuv = wub[:].rearrange("p (a e) -> p a e", e=128)
                    ps = ffps.tile([128, 512], f32, tag="ps", name="ps")
                    for dt in range(16):
                        nc.tensor.matmul(ps[:], wuv[:, dt, :],
                                         h2f[:, dt, :],
                                         start=(dt == 0), stop=(dt == 15))
                    tg = gtp.tile([128, 512], f32, tag="tg", name="tg")
                    nc.vector.tensor_tensor(out=tg, in0=ps[:], in1=scc_bc,
                                            op=Alu.mult)
                    ug = gtp.tile([128, 512], f32, tag="ug", name="ug")
                    nc.scalar.activation(out=ug, in_=tg, func=Act.Gelu,
                                         scale=1.0)
                    nc.sync.dma_start(
                        out=u_dram[fc * 128:(fc + 1) * 128, :], in_=ug)
                    prt = gtp.tile([128, 512], f32, tag="prt", name="prt")
                    nc.gpsimd.partition_all_reduce(prt[:], ug[:],
                                                   channels=128,
                                                   reduce_op=Red.absmax)
                    if fc == 0:
                        nc.vector.tensor_copy(out=uamax, in_=prt[0:1, :])
                    else:
                        nc.vector.tensor_tensor(out=uamax, in0=uamax,
                                                in1=prt[0:1, :],
                                                op=Alu.max)
            rqu_bc = ubcp.tile([128, 512], f32, name="rqu_bc")
            usc_bc = ubcp.tile([128, 512], f32, name="usc_bc")
            row_scales(ubcp, uamax[0:1, :], rqu_bc, usc_bc)
            Uq = [uqp.tile([128, 512], bf16, name=f"uq{i}")
                  for i in range(64)]
            with ExitStack() as s6q:
                ulp = s6q.enter_context(tc.tile_pool(name="ulp", bufs=3))
                for fc in range(64):
                    uf = ulp.tile([128, 512], f32, tag="uf", name="uf")
                    nc.sync.dma_start(
                        out=uf, in_=u_dram[fc * 128:(fc + 1) * 128, :])
                    tq = ulp.tile([128, 512], f32, tag="tq", name="tq")
                    nc.vector.tensor_tensor(out=tq, in0=uf[:], in1=rqu_bc,
                                            op=Alu.mult)
                    nc.vector.tensor_scalar(out=Uq[fc], in0=tq,
                                            scalar1=MAGIC, scalar2=MAGIC,
                                            op0=Alu.add, op1=Alu.subtract)

            # ---- P7: FFN down + residual -> out ----
            with ExitStack() as s7:
                wdp = s7.enter_context(tc.tile_pool(name="wdp", bufs=2))
                drp = s7.enter_context(tc.tile_pool(name="drp", bufs=2))
                dps = s7.enter_context(tc.tile_pool(name="dps", bufs=4,
                                                    space="PSUM"))
                agwd_v = agwd_out[:].rearrange("(r c p) f -> r c p f",
                                               c=2, p=128)
                for oc in range(16):
                    wdb = wdp.tile([128, 8192], bf16, tag="wd", name="wd")
                    nc.scalar.dma_start(
                        out=wdb[:].rearrange("p (r k) -> p r k", r=8),
                        in_=agwd_v[:, oc // 8, :,
                                   (oc % 8) * 1024:(oc % 8 + 1) * 1024]
                        .rearrange("r p k -> p r k"))
                    wdv = wdb[:].rearrange("p (a e) -> p a e", e=128)
                    ps = dps.tile([128, 512], f32, tag="ps", name="ps")
                    for fc in range(64):
                        nc.tensor.matmul(ps[:], wdv[:, fc, :], Uq[fc][:],
                                         start=(fc == 0), stop=(fc == 63))
                    x2r = drp.tile([128, 512], f32, tag="x2r", name="x2r")
                    nc.sync.dma_start(
                        out=x2r, in_=x2t_dram[oc * 128:(oc + 1) * 128, :])
                    t1 = drp.tile([128, 512], f32, tag="t1", name="t1")
                    nc.vector.tensor_tensor(out=t1, in0=ps[:], in1=usc_bc,
                                            op=Alu.mult)
                    t2 = drp.tile([128, 512], f32, tag="t2", name="t2")
                    nc.vector.tensor_tensor(out=t2, in0=t1, in1=x2r,
                                            op=Alu.add)
                    nc.sync.dma_start(
                        out=out_d[oc * 128:(oc + 1) * 128, :], in_=t2)

    nc.finalize()
    return nc


_CACHE = {}


def _pack_weights(wq, wk, wv, wo, wu, wd, r):
    # pure layout transforms (slice / transpose / reshape) — no arithmetic
    def pack_qkv(w):
        s = w[r * 256:(r + 1) * 256, :].T            # [2048, 256]
        return np.ascontiguousarray(
            s.reshape(16, 128, 256).transpose(1, 0, 2).reshape(128, 4096))

    def pack_wo(w):
        s = w.T[r * 256:(r + 1) * 256, :]            # [256, 2048]
        return np.ascontiguousarray(
            s.reshape(2, 128, 2048).transpose(1, 0, 2).reshape(128, 4096))

    def pack_wu(w):
        s = w[r * 1024:(r + 1) * 1024, :].T          # [2048, 1024]
        s = s.reshape(16, 128, 8, 128)               # [dblk, p, lc, fb]
        return np.ascontiguousarray(
            s.transpose(1, 2, 0, 3).reshape(128, 16384))

    def pack_wd(w):
        s = w.T[r * 1024:(r + 1) * 1024, :]          # [1024, 2048]
        s = s.reshape(8, 128, 16, 128)               # [fblk, p, oc, od]
        return np.ascontiguousarray(
            s.transpose(1, 2, 0, 3).reshape(128, 16384))

    return (pack_qkv(wq), pack_qkv(wk), pack_qkv(wv), pack_wo(wo),
            pack_wu(wu), pack_wd(wd))


def kernel(**inputs):
    x = np.asarray(inputs["x"], dtype=np.float32)
    wq = np.asarray(inputs["wq"], dtype=np.float32)
    wk = np.asarray(inputs["wk"], dtype=np.float32)
    wv = np.asarray(inputs["wv"], dtype=np.float32)
    wo = np.asarray(inputs["wo"], dtype=np.float32)
    wu = np.asarray(inputs["wu"], dtype=np.float32)
    wd = np.asarray(inputs["wd"], dtype=np.float32)

    if "nc" not in _CACHE:
        _CACHE["nc"] = build_graph()
    nc = _CACHE["nc"]

    mask = np.zeros((4, 128, 512), dtype=np.float32)
    jj = np.arange(512)[None, :]
    ii = np.arange(128)[:, None]
    for m in range(4):
        mask[m] = np.where(jj <= m * 128 + ii, 0.0, NEG)

    in_maps = []
    for r in range(NCORES):
        b, c = r // 4, r % 4
        xr = x[b, c * 512:(c + 1) * 512, :]
        wqp, wkp, wvp, wop, wup, wdp = _pack_weights(
            wq, wk, wv, wo, wu, wd, r)
        in_maps.append({
            "x": np.ascontiguousarray(xr),
            "xT": np.ascontiguousarray(xr.T),
            "wqT": wqp, "wkT": wkp, "wvT": wvp, "woT": wop,
            "wuT": wup, "wdT": wdp,
            "mask": mask,
        })

    from concourse import bass_utils
    res = bass_utils.run_bass_kernel_spmd(
        nc, in_maps, core_ids=list(range(NCORES)))
    _CACHE["last_result"] = res

    out = np.zeros((B, T, D), dtype=np.float32)
    for r in range(NCORES):
        b, c = r // 4, r % 4
        out[b, c * 512:(c + 1) * 512, :] = res.results[r]["out"].T
    return out


if __name__ == "__main__":
    nc = build_graph()
    n_inst = sum(len(bb.instructions) for bb in nc.main_func.blocks)
    print("graph built ok, instructions:", n_inst)
